# revision 30
# baseline (speedup 1.0000x reference)
"""nn_MultiHeadAttention (B=2, S=2048, D=2048, H=16) on 8 NeuronCores.

The reference module splits heads with a plain reshape (no transpose):
    Q = (x @ Wq.T).reshape(B, H, S, Dh)
so head h attends over ROWS [128h, 128h+128) of Qmat = x @ Wq.T, with
attention position s = 16a + r mapping to (row 128h + a, feature slice
[128r, 128r+128)).  The merge DOES transpose (standard), so
    y = sum_h outh @ Wo[:, 128h:128h+128].T.

Sharding: core c handles batch b=c//4 and head-group g=c%4 (heads
4g..4g+3, i.e. Qmat/Kmat/Vmat rows [512g, 512g+512) of its batch).  Each
core computes those projection row-slices (against the FULL Wq/Wk/Wv),
causal attention in the scrambled index space, and a partial output
projection against its column slice of Wo.  The host sums the 4 bf16
partials per batch in fp32.

v2 changes vs the bf16 baseline (385-460us):
  - Q/K projections run in fp8(e4m3) with DoubleRow perf mode: weights
    pre-scaled x32 into fp8 range on the host, the x1024 score scale is
    folded into the softmax exp.  Contraction pairs of 128-deep k-tiles
    go through the PE at 2 fp8 MACs/cell/cycle.
  - The PSUM->SBUF scatter copies of phase A (2B-strided, ~960ns each on
    the scalar engine) are round-robined across scalar and vector so
    neither engine paces the PE.
  - exp runs once per k-octet PAIR over a 2-bank PSUM tile [128, 1024],
    halving activation instruction overhead.
  - startup DMA is chunked so the first matmul waits on ~256KB, not 3MB.
"""

import sys

try:
    import concourse.bass as bass
except ImportError:  # harness may not have the repo on PYTHONPATH
    for p in ("/root/.axon_site", "/root/.axon_site/_ro/trn_rl_repo",
              "/root/.axon_site/_ro/pypackages", "/opt/trn_rl_repo"):
        if p not in sys.path:
            sys.path.append(p)
    import concourse.bass as bass

import numpy as np

import concourse.mybir as mybir
import concourse.tile as tile
from concourse.bass_utils import run_bass_kernel_spmd

F32 = mybir.dt.float32
BF16 = mybir.dt.bfloat16
FP8 = mybir.dt.float8e4
DT = BF16
AF = mybir.ActivationFunctionType
DR = mybir.MatmulPerfMode.DoubleRow

B = 2
S = 2048
DM = 2048
H = 16
DH = 128
N_CORES = 8
HPC = 4                 # heads per core
DL = HPC * DH           # 512: per-core row/col slice width
P = 128
QB = 512                # q-block width = 32 a x 16 r
N_DM = DM // P          # 16 contraction tiles
NR = 16                 # r-stripes per head
WSC = 32.0              # fp8 weight pre-scale for Wq/Wk
ESC = 1.0 / (DH * WSC * WSC)   # exp scale: undo x32 x32 and /128


def _split_multi_waits(nc):
    """This container's walrus rejects >1 sync-wait per instruction.
    Hoist extra waits onto same-engine NoOps inserted just before."""
    ctr = 0
    for f in nc.m.functions:
        for bb in f.blocks:
            insts = bb.instructions
            fixes = []
            for idx, inst in enumerate(insts):
                si = inst.sync_info
                ow = list(si.on_wait) if si and si.on_wait else []
                if len(ow) > 1:
                    fixes.append((idx, inst, ow, si))
            for idx, inst, ow, si in reversed(fixes):
                inst.sync_info = mybir.SyncInfo(on_wait=ow[-1:], on_update=si.on_update)
                for w in reversed(ow[:-1]):
                    ctr += 1
                    nop = mybir.InstNoOp(
                        name=f"I-waitsplit-{ctr}", engine=inst.engine, ins=[], outs=[]
                    )
                    nop.sync_info = mybir.SyncInfo(on_wait=[w], on_update=[])
                    nc.register_instruction(nop, overwrite=True)
                    insts.insert(idx, nop)
    return ctr


def _build_nc():
    nc = bass.Bass(target_bir_lowering=False)

    xsb_d = nc.dram_tensor("xsb", [DM, DL], BF16, kind="ExternalInput")   # x[b,rows].T
    xs8_d = nc.dram_tensor("xs8", [DM, DL], FP8, kind="ExternalInput")
    wvt_d = nc.dram_tensor("wvt", [DM, DM], BF16, kind="ExternalInput")   # Wv.T
    wkt_d = nc.dram_tensor("wkt8", [DM, DM], FP8, kind="ExternalInput")   # Wk.T * 32
    wqt_d = nc.dram_tensor("wqt8", [DM, DM], FP8, kind="ExternalInput")   # Wq.T * 32
    wot_d = nc.dram_tensor("wot", [DL, DM], BF16, kind="ExternalInput")   # Wo[:,sl].T
    mask_d = nc.dram_tensor("maskc", [4, P, QB], BF16, kind="ExternalInput")
    ones_d = nc.dram_tensor("ones", [P, P], BF16, kind="ExternalInput")
    ident_d = nc.dram_tensor("ident", [P, P], BF16, kind="ExternalInput")
    yt_d = nc.dram_tensor("yt", [DM, S], BF16, kind="ExternalOutput")     # partial y[b].T

    yt_t3 = yt_d.rearrange("(o p) s -> p o s", p=P)

    # alternate the strided scatter copies between scalar and vector
    _cp = [0]

    def scatter_copy(dst, src):
        eng = nc.scalar.copy if _cp[0] % 2 == 0 else nc.vector.tensor_copy
        _cp[0] += 1
        eng(dst, src)

    with tile.TileContext(nc) as tc:
        with (
            tc.tile_pool(name="stage", bufs=4) as stage,
            tc.tile_pool(name="small", bufs=2) as small,
            tc.tile_pool(name="proj", bufs=HPC) as proj,
            tc.tile_pool(name="ps_s2", bufs=2, space="PSUM") as ps_s2,
            tc.tile_pool(name="ps_o", bufs=1, space="PSUM") as ps_o,
            tc.tile_pool(name="ps_l", bufs=1, space="PSUM") as ps_l,
            tc.tile_pool(name="ps_t", bufs=2, space="PSUM") as ps_t,
            nc.allow_low_precision(reason="bf16/fp8 attention kernel"),
        ):
            # per-head projection tiles in [dh, a, r] layout; phase A scatters
            # r-stripes into them (strided copies, split across scalar+vector)
            qt2 = [proj.tile([P, P, NR], DT, tag="qt2", name=f"qt2_{i}") for i in range(HPC)]
            kt2 = [proj.tile([P, P, NR], DT, tag="kt2", name=f"kt2_{i}") for i in range(HPC)]
            vt2 = [proj.tile([P, P, NR], DT, tag="vt2", name=f"vt2_{i}") for i in range(HPC)]

            # ---- phase A: projection row-slices straight into SBUF ----
            with (
                tc.tile_pool(name="xpool", bufs=1) as xpool,
                tc.tile_pool(name="wvp", bufs=3) as wvp,
                tc.tile_pool(name="w8p", bufs=4) as w8p,
            ):
                xb_t = xpool.tile([P, N_DM, DL], BF16, tag="xb")
                x8_t = xpool.tile([P, N_DM, DL], FP8, tag="x8")
                xb_t3 = xsb_d.rearrange("(o p) s -> p o s", p=P)
                x8_t3 = xs8_d.rearrange("(o p) s -> p o s", p=P)
                wv_t3 = wvt_d.rearrange("(o p) d -> p o d", p=P)

                # first V weight tile + first half of x ship before anything
                # else so the first matmul can start ~6us in
                wv0 = wvp.tile([P, N_DM, P], BF16, tag="wv")
                nc.sync.dma_start(wv0[:, :4, :], wv_t3[:, :4, 0:P])
                nc.sync.dma_start(xb_t[:, :4, :], xb_t3[:, :4, :])
                nc.sync.dma_start(wv0[:, 4:, :], wv_t3[:, 4:, 0:P])
                nc.sync.dma_start(xb_t[:, 4:8, :], xb_t3[:, 4:8, :])
                nc.sync.dma_start(xb_t[:, 8:12, :], xb_t3[:, 8:12, :])
                nc.sync.dma_start(xb_t[:, 12:, :], xb_t3[:, 12:, :])

                # The fp8-DR passes finish each PSUM group in ~2.3us but the
                # 4 strided scatter copies take ~2us on the drain engines, so
                # a 2-deep PSUM ring has no slack.  Round-robin over 4 DISTINCT
                # tiles drawn from the shared pools (PSUM matmul groups
                # serialize at tile granularity, so two iterations must not
                # share a tile; and reusing phase B's pools here avoids a
                # pool-scope drain barrier at the A->B boundary).

                def a_psum(rt):
                    r = rt % 4
                    if r < 2:
                        return ps_s2.tile([P, 2, QB], F32, tag="ps2",
                                          name="apsum2")[:, 0, :]
                    if r == 2:
                        return ps_o.tile([P, QB], F32, tag="po", name="apsumo")[:]
                    return ps_l.tile([P, QB], F32, tag="pl", name="apsuml")[:]

                # V pass (bf16)
                for rt in range(NR):
                    if rt == 0:
                        w_t = wv0
                    else:
                        w_t = wvp.tile([P, N_DM, P], BF16, tag="wv")
                        nc.sync.dma_start(w_t[:], wv_t3[:, :, rt * P:(rt + 1) * P])
                    psum = a_psum(rt)
                    for dm in range(N_DM):
                        nc.tensor.matmul(
                            psum, lhsT=w_t[:, dm, :], rhs=xb_t[:, dm, :],
                            start=(dm == 0), stop=(dm == N_DM - 1),
                        )
                    for hl in range(HPC):
                        scatter_copy(
                            vt2[hl][:, :, rt], psum[:, hl * P:(hl + 1) * P]
                        )
                    if rt == 1:
                        # x8 is only needed by the K pass; ship it mid-V-pass
                        nc.sync.dma_start(x8_t[:, :8, :], x8_t3[:, :8, :])
                        nc.sync.dma_start(x8_t[:, 8:, :], x8_t3[:, 8:, :])

                # K then Q passes (fp8 DoubleRow over contraction pairs)
                for w_d, dst in ((wkt_d, kt2), (wqt_d, qt2)):
                    w_t3 = w_d.rearrange("(o p) d -> p o d", p=P)
                    for rt in range(NR):
                        w_t = w8p.tile([P, N_DM, P], FP8, tag="w8")
                        nc.sync.dma_start(w_t[:], w_t3[:, :, rt * P:(rt + 1) * P])
                        psum = a_psum(rt)
                        for dp in range(8):
                            nc.tensor.matmul(
                                psum,
                                lhsT=w_t[:, 2 * dp:2 * dp + 2, :],
                                rhs=x8_t[:, 2 * dp:2 * dp + 2, :],
                                start=(dp == 0), stop=(dp == 7),
                                perf_mode=DR,
                            )
                        for hl in range(HPC):
                            scatter_copy(
                                dst[hl][:, :, rt], psum[:, hl * P:(hl + 1) * P]
                            )

            # ---- phase B+C: attention per head, with the output projection
            # interleaved one q-block behind (its matmuls soak up the PE while
            # the DVE normalize / scalar exp tails drain) ----
            # k-octet m covers kidx = a''*16 + r' (a'' in [8m,8m+8));
            # q-block qb covers qidx = a*16 + r (a in [32qb, 32qb+32)).
            with (
                tc.tile_pool(name="bconst", bufs=1) as bconst,
                tc.tile_pool(name="hpool", bufs=HPC) as hpool,
                tc.tile_pool(name="atpool", bufs=4) as atpool,
                tc.tile_pool(name="attt", bufs=HPC) as attt_pool,
                tc.tile_pool(name="wop", bufs=1) as wop,
            ):
                ones_t = bconst.tile([P, P], DT, tag="ones")
                nc.sync.dma_start(ones_t[:], ones_d[:])
                mask_t = bconst.tile([P, 4, QB], BF16, tag="mask")
                nc.sync.dma_start(mask_t[:], mask_d.rearrange("c p q -> p c q"))
                ident_t = bconst.tile([P, P], BF16, tag="ident")
                nc.sync.dma_start(ident_t[:], ident_d[:])
                wot_t = wop.tile([P, HPC, DM], BF16, tag="wo")
                nc.sync.dma_start(wot_t[:, :2, :],
                                  wot_d.rearrange("(hl p) d -> p hl d", p=P)[:, :2, :])
                nc.sync.dma_start(wot_t[:, 2:, :],
                                  wot_d.rearrange("(hl p) d -> p hl d", p=P)[:, 2:, :])

                vk_tiles = [hpool.tile([P, NR, P], DT, tag="v", name=f"vk_{i}")
                            for i in range(HPC)]
                att_tiles = [attt_pool.tile([P, P, NR], DT, tag="attT",
                                            name=f"att_{i}")
                             for i in range(HPC)]
                att_flat = [t[:].rearrange("p a r -> p (a r)") for t in att_tiles]

                def attention_block(hl, qb):
                    vk_h = vk_tiles[hl]
                    att_h = att_tiles[hl]
                    a0 = 32 * qb
                    nk = 4 * (qb + 1)   # k-octets 0..nk-1
                    npair = nk // 2
                    psum_o = ps_o.tile([P, QB], F32, tag="po", name="psum_o")
                    psum_l = ps_l.tile([P, QB], F32, tag="pl", name="psum_l")
                    ats = [None] * npair

                    def emit_vk(m):
                        pst = ps_t.tile([P, P], DT, tag="pt", name="pst")
                        nc.tensor.transpose(
                            pst[:], vt2[hl][:, 8 * m:8 * (m + 1), :], ident_t[:]
                        )
                        nc.scalar.copy(vk_h[:, m, :], pst[:])

                    def emit_scores(pr):
                        psum_s = ps_s2.tile([P, 2, QB], F32, tag="ps2",
                                            name="psum_s")
                        for j in (0, 1):
                            m = 2 * pr + j
                            nc.tensor.matmul(
                                psum_s[:, j, :],
                                lhsT=kt2[hl][:, 8 * m:8 * (m + 1), :],
                                rhs=qt2[hl][:, a0:a0 + 32, :],
                                start=True, stop=True,
                            )
                        atp = atpool.tile([P, 2, QB], DT, tag="at", name="atp")
                        nc.scalar.activation(atp[:], psum_s[:], AF.Exp, scale=ESC)
                        for j in (0, 1):
                            m = 2 * pr + j
                            if m >= 4 * qb:
                                nc.vector.tensor_mul(
                                    atp[:, j, :], atp[:, j, :],
                                    mask_t[:, m - 4 * qb, :],
                                )
                        ats[pr] = atp

                    def emit_ov(pr):
                        for j in (0, 1):
                            m = 2 * pr + j
                            nc.tensor.matmul(
                                psum_o[:],
                                lhsT=vk_h[:, m, :], rhs=ats[pr][:, j, :],
                                start=(m == 0), stop=(m == nk - 1),
                            )
                            nc.tensor.matmul(
                                psum_l[:],
                                lhsT=ones_t[:], rhs=ats[pr][:, j, :],
                                start=(m == 0), stop=(m == nk - 1),
                            )

                    for m in range(4 * qb, nk):
                        emit_vk(m)
                    emit_scores(0)
                    for pr in range(1, npair):
                        emit_scores(pr)
                        emit_ov(pr - 1)
                    emit_ov(npair - 1)

                    # normalize: att = psum_o * (1/l).  Both PSUM tiles are
                    # copied out to SBUF immediately so their single banks
                    # free before the ~3.3us reciprocal runs (else the next
                    # block's first ov/ones matmuls stall the PE).
                    obuf = small.tile([P, QB], F32, tag="obuf")
                    nc.vector.tensor_copy(obuf[:], psum_o[:])
                    lbuf = small.tile([P, QB], F32, tag="lbuf")
                    nc.vector.tensor_copy(lbuf[:], psum_l[:])
                    rcb = small.tile([P, QB], F32, tag="rcb")
                    nc.vector.reciprocal(rcb[:], lbuf[:])
                    nc.vector.tensor_mul(
                        att_h[:, a0:a0 + 32, :],
                        obuf[:].rearrange("p (a r) -> p a r", a=32),
                        rcb[:].rearrange("p (a r) -> p a r", a=32),
                    )

                def out_chunk(sb):
                    # y.T columns [512*sb, 512*sb+512) for all 16 row-blocks
                    for ot in range(N_DM):
                        psum = ps_s2.tile([P, 2, QB], F32, tag="ps2",
                                          name="cpsum")[:, 0, :]
                        for hl in range(HPC):
                            nc.tensor.matmul(
                                psum,
                                lhsT=wot_t[:, hl, ot * P:(ot + 1) * P],
                                rhs=att_flat[hl][:, sb * QB:(sb + 1) * QB],
                                start=(hl == 0), stop=(hl == HPC - 1),
                            )
                        st = stage.tile([P, QB], DT, tag="ystage")
                        nc.any.tensor_copy(st[:], psum)
                        nc.sync.dma_start(yt_t3[:, ot, sb * QB:(sb + 1) * QB],
                                          st[:])

                for qb in range(4):
                    for hl in range(HPC):
                        attention_block(hl, qb)
                        if hl == 0 and qb > 0:
                            out_chunk(qb - 1)
                out_chunk(3)

    _split_multi_waits(nc)
    return nc


_NC = None


def _make_masks():
    # a-blocked causal masks for diagonal tiles, (a-outer, r-inner) order:
    # k partition index p = a''*16 + r';  q column index j = a_rel*16 + r
    # allow k <= q:  16*(8*mi + a'') + r'  <=  16*a_rel + r
    k_lin = (16 * np.arange(8)[:, None] + np.arange(NR)[None, :]).reshape(-1)   # 128
    q_lin = (16 * np.arange(32)[:, None] + np.arange(NR)[None, :]).reshape(-1)  # 512
    out = np.empty((4, P, QB), dtype=np.float32)
    for mi in range(4):
        out[mi] = ((k_lin[:, None] + 128 * mi) <= q_lin[None, :]).astype(np.float32)
    return out


def kernel(x, Wq, Wk, Wv, Wo, _want_trace=False, **_trace_kw):
    global _NC
    if _NC is None:
        _NC = _build_nc()
    nc = _NC

    import ml_dtypes
    bf16 = ml_dtypes.bfloat16
    fp8 = ml_dtypes.float8_e4m3fn

    def to8(arr, scale):
        return np.clip(np.asarray(arr, np.float32) * scale, -240.0, 240.0).astype(fp8)

    x = np.asarray(x, dtype=np.float32)
    Wq = np.asarray(Wq, dtype=np.float32)
    Wk = np.asarray(Wk, dtype=np.float32)
    Wv = np.asarray(Wv, dtype=np.float32)
    Wo = np.asarray(Wo, dtype=np.float32)
    wqt8 = to8(np.ascontiguousarray(Wq.T), WSC)
    wkt8 = to8(np.ascontiguousarray(Wk.T), WSC)
    wvt = np.ascontiguousarray(Wv.T).astype(bf16)
    masks = _make_masks().astype(bf16)
    ones = np.ones((P, P), dtype=bf16)
    ident = np.eye(P, dtype=np.float32).astype(bf16)

    in_maps = []
    for c in range(N_CORES):
        b, g = divmod(c, HPC)
        sl = slice(g * DL, (g + 1) * DL)
        xsT = np.ascontiguousarray(x[b, sl, :].T)
        in_maps.append({
            "xsb": xsT.astype(bf16),
            "xs8": to8(xsT, 1.0),
            "wvt": wvt,
            "wkt8": wkt8,
            "wqt8": wqt8,
            "wot": np.ascontiguousarray(Wo[:, sl].T).astype(bf16),
            "maskc": masks,
            "ones": ones,
            "ident": ident,
        })

    res = run_bass_kernel_spmd(
        nc, in_maps, list(range(N_CORES)),
        trace=_want_trace, **_trace_kw,
    )

    y = np.empty((B, S, DM), dtype=np.float32)
    for b in range(B):
        acc = res.results[HPC * b]["yt"].astype(np.float32)
        for g in range(1, HPC):
            acc += res.results[HPC * b + g]["yt"].astype(np.float32)
        y[b] = acc.T
    if _want_trace:
        return y, res
    return y


# revision 31
# speedup vs baseline: 1.0465x; 1.0465x over previous
"""nn_MultiHeadAttention (B=2, S=2048, D=2048, H=16) on 8 NeuronCores.

The reference module splits heads with a plain reshape (no transpose):
    Q = (x @ Wq.T).reshape(B, H, S, Dh)
so head h attends over ROWS [128h, 128h+128) of Qmat = x @ Wq.T, with
attention position s = 16a + r mapping to (row 128h + a, feature slice
[128r, 128r+128)).  The merge DOES transpose (standard), so
    y = sum_h outh @ Wo[:, 128h:128h+128].T.

Sharding: core c handles batch b=c//4 and head-group g=c%4 (heads
4g..4g+3, i.e. Qmat/Kmat/Vmat rows [512g, 512g+512) of its batch).  Each
core computes those projection row-slices (against the FULL Wq/Wk/Wv),
causal attention in the scrambled index space, and a partial output
projection against its column slice of Wo.  The host sums the 4 bf16
partials per batch in fp32.

v2 changes vs the bf16 baseline (385-460us):
  - Q/K projections run in fp8(e4m3) with DoubleRow perf mode: weights
    pre-scaled x32 into fp8 range on the host, the x1024 score scale is
    folded into the softmax exp.  Contraction pairs of 128-deep k-tiles
    go through the PE at 2 fp8 MACs/cell/cycle.
  - The PSUM->SBUF scatter copies of phase A (2B-strided, ~960ns each on
    the scalar engine) are round-robined across scalar and vector so
    neither engine paces the PE.
  - exp runs once per k-octet PAIR over a 2-bank PSUM tile [128, 1024],
    halving activation instruction overhead.
  - startup DMA is chunked so the first matmul waits on ~256KB, not 3MB.
"""

import sys

try:
    import concourse.bass as bass
except ImportError:  # harness may not have the repo on PYTHONPATH
    for p in ("/root/.axon_site", "/root/.axon_site/_ro/trn_rl_repo",
              "/root/.axon_site/_ro/pypackages", "/opt/trn_rl_repo"):
        if p not in sys.path:
            sys.path.append(p)
    import concourse.bass as bass

import numpy as np

import concourse.mybir as mybir
import concourse.tile as tile
from concourse.bass_utils import run_bass_kernel_spmd

F32 = mybir.dt.float32
BF16 = mybir.dt.bfloat16
FP8 = mybir.dt.float8e4
DT = BF16
AF = mybir.ActivationFunctionType
DR = mybir.MatmulPerfMode.DoubleRow

B = 2
S = 2048
DM = 2048
H = 16
DH = 128
N_CORES = 8
HPC = 4                 # heads per core
DL = HPC * DH           # 512: per-core row/col slice width
P = 128
QB = 512                # q-block width = 32 a x 16 r
N_DM = DM // P          # 16 contraction tiles
NR = 16                 # r-stripes per head
WSC = 32.0              # fp8 weight pre-scale for Wq/Wk
ESC = 1.0 / (DH * WSC * WSC)   # exp scale: undo x32 x32 and /128


def _split_multi_waits(nc):
    """This container's walrus rejects >1 sync-wait per instruction.
    Hoist extra waits onto same-engine NoOps inserted just before."""
    ctr = 0
    for f in nc.m.functions:
        for bb in f.blocks:
            insts = bb.instructions
            fixes = []
            for idx, inst in enumerate(insts):
                si = inst.sync_info
                ow = list(si.on_wait) if si and si.on_wait else []
                if len(ow) > 1:
                    fixes.append((idx, inst, ow, si))
            for idx, inst, ow, si in reversed(fixes):
                inst.sync_info = mybir.SyncInfo(on_wait=ow[-1:], on_update=si.on_update)
                for w in reversed(ow[:-1]):
                    ctr += 1
                    nop = mybir.InstNoOp(
                        name=f"I-waitsplit-{ctr}", engine=inst.engine, ins=[], outs=[]
                    )
                    nop.sync_info = mybir.SyncInfo(on_wait=[w], on_update=[])
                    nc.register_instruction(nop, overwrite=True)
                    insts.insert(idx, nop)
    return ctr


def _build_nc():
    nc = bass.Bass(target_bir_lowering=False)

    xsb_d = nc.dram_tensor("xsb", [DM, DL], BF16, kind="ExternalInput")   # x[b,rows].T
    xs8_d = nc.dram_tensor("xs8", [DM, DL], FP8, kind="ExternalInput")
    wvt_d = nc.dram_tensor("wvt", [DM, DM], BF16, kind="ExternalInput")   # Wv.T
    wkt_d = nc.dram_tensor("wkt8", [DM, DM], FP8, kind="ExternalInput")   # Wk.T * 32
    wqt_d = nc.dram_tensor("wqt8", [DM, DM], FP8, kind="ExternalInput")   # Wq.T * 32
    wot_d = nc.dram_tensor("wot", [DL, DM], BF16, kind="ExternalInput")   # Wo[:,sl].T
    mask_d = nc.dram_tensor("maskc", [4, P, QB], BF16, kind="ExternalInput")
    ones_d = nc.dram_tensor("ones", [P, P], BF16, kind="ExternalInput")
    ident_d = nc.dram_tensor("ident", [P, P], BF16, kind="ExternalInput")
    yt_d = nc.dram_tensor("yt", [DM, S], BF16, kind="ExternalOutput")     # partial y[b].T

    yt_t3 = yt_d.rearrange("(o p) s -> p o s", p=P)

    # alternate the strided scatter copies between scalar and vector
    _cp = [0]

    def scatter_copy(dst, src):
        eng = nc.scalar.copy if _cp[0] % 2 == 0 else nc.vector.tensor_copy
        _cp[0] += 1
        eng(dst, src)

    with tile.TileContext(nc) as tc:
        with (
            tc.tile_pool(name="stage", bufs=4) as stage,
            tc.tile_pool(name="small", bufs=2) as small,
            tc.tile_pool(name="proj", bufs=HPC) as proj,
            tc.tile_pool(name="ps_s2", bufs=2, space="PSUM") as ps_s2,
            tc.tile_pool(name="ps_o", bufs=1, space="PSUM") as ps_o,
            tc.tile_pool(name="ps_l", bufs=1, space="PSUM") as ps_l,
            tc.tile_pool(name="ps_t", bufs=2, space="PSUM") as ps_t,
            nc.allow_low_precision(reason="bf16/fp8 attention kernel"),
        ):
            # per-head projection tiles in [dh, a, r] layout; phase A scatters
            # r-stripes into them (strided copies, split across scalar+vector)
            qt2 = [proj.tile([P, P, NR], DT, tag="qt2", name=f"qt2_{i}") for i in range(HPC)]
            kt2 = [proj.tile([P, P, NR], DT, tag="kt2", name=f"kt2_{i}") for i in range(HPC)]
            vt2 = [proj.tile([P, P, NR], DT, tag="vt2", name=f"vt2_{i}") for i in range(HPC)]

            # ---- phase A: projection row-slices straight into SBUF ----
            with (
                tc.tile_pool(name="xpool", bufs=1) as xpool,
                tc.tile_pool(name="wvp", bufs=3) as wvp,
                tc.tile_pool(name="w8p", bufs=4) as w8p,
            ):
                xb_t = xpool.tile([P, N_DM, DL], BF16, tag="xb")
                x8_t = xpool.tile([P, N_DM, DL], FP8, tag="x8")
                xb_t3 = xsb_d.rearrange("(o p) s -> p o s", p=P)
                x8_t3 = xs8_d.rearrange("(o p) s -> p o s", p=P)
                wv_t3 = wvt_d.rearrange("(o p) d -> p o d", p=P)

                # first V weight tile + first half of x ship before anything
                # else so the first matmul can start ~6us in
                wv0 = wvp.tile([P, N_DM, P], BF16, tag="wv")
                nc.sync.dma_start(wv0[:, :4, :], wv_t3[:, :4, 0:P])
                nc.sync.dma_start(xb_t[:, :4, :], xb_t3[:, :4, :])
                nc.sync.dma_start(wv0[:, 4:, :], wv_t3[:, 4:, 0:P])
                nc.sync.dma_start(xb_t[:, 4:8, :], xb_t3[:, 4:8, :])
                nc.sync.dma_start(xb_t[:, 8:12, :], xb_t3[:, 8:12, :])
                nc.sync.dma_start(xb_t[:, 12:, :], xb_t3[:, 12:, :])

                # The fp8-DR passes finish each PSUM group in ~2.3us but the
                # 4 strided scatter copies take ~2us on the drain engines, so
                # a 2-deep PSUM ring has no slack.  Round-robin over 4 DISTINCT
                # tiles drawn from the shared pools (PSUM matmul groups
                # serialize at tile granularity, so two iterations must not
                # share a tile; and reusing phase B's pools here avoids a
                # pool-scope drain barrier at the A->B boundary).

                def a_psum(rt):
                    r = rt % 4
                    if r < 2:
                        return ps_s2.tile([P, 2, QB], F32, tag="ps2",
                                          name="apsum2")[:, 0, :]
                    if r == 2:
                        return ps_o.tile([P, QB], F32, tag="po", name="apsumo")[:]
                    return ps_l.tile([P, QB], F32, tag="pl", name="apsuml")[:]

                # V pass (bf16)
                for rt in range(NR):
                    if rt == 0:
                        w_t = wv0
                    else:
                        w_t = wvp.tile([P, N_DM, P], BF16, tag="wv")
                        nc.sync.dma_start(w_t[:], wv_t3[:, :, rt * P:(rt + 1) * P])
                    psum = a_psum(rt)
                    for dm in range(N_DM):
                        nc.tensor.matmul(
                            psum, lhsT=w_t[:, dm, :], rhs=xb_t[:, dm, :],
                            start=(dm == 0), stop=(dm == N_DM - 1),
                        )
                    for hl in range(HPC):
                        scatter_copy(
                            vt2[hl][:, :, rt], psum[:, hl * P:(hl + 1) * P]
                        )
                    if rt == 1:
                        # x8 is only needed by the K pass; ship it mid-V-pass
                        nc.sync.dma_start(x8_t[:, :8, :], x8_t3[:, :8, :])
                        nc.sync.dma_start(x8_t[:, 8:, :], x8_t3[:, 8:, :])

                # K then Q passes (fp8 DoubleRow over contraction pairs)
                for w_d, dst in ((wkt_d, kt2), (wqt_d, qt2)):
                    w_t3 = w_d.rearrange("(o p) d -> p o d", p=P)
                    for rt in range(NR):
                        w_t = w8p.tile([P, N_DM, P], FP8, tag="w8")
                        nc.sync.dma_start(w_t[:], w_t3[:, :, rt * P:(rt + 1) * P])
                        psum = a_psum(rt)
                        for dp in range(8):
                            nc.tensor.matmul(
                                psum,
                                lhsT=w_t[:, 2 * dp:2 * dp + 2, :],
                                rhs=x8_t[:, 2 * dp:2 * dp + 2, :],
                                start=(dp == 0), stop=(dp == 7),
                                perf_mode=DR,
                            )
                        for hl in range(HPC):
                            scatter_copy(
                                dst[hl][:, :, rt], psum[:, hl * P:(hl + 1) * P]
                            )

            # ---- phase B: attention per head (scrambled index space) ----
            # k-octet m covers kidx = a''*16 + r' (a'' in [8m,8m+8));
            # q-block qb covers qidx = a*16 + r (a in [32qb, 32qb+32)).
            with (
                tc.tile_pool(name="bconst", bufs=1) as bconst,
                tc.tile_pool(name="hpool", bufs=3) as hpool,
                tc.tile_pool(name="atpool", bufs=4) as atpool,
                tc.tile_pool(name="attt", bufs=HPC) as attt_pool,
            ):
                ones_t = bconst.tile([P, P], DT, tag="ones")
                nc.sync.dma_start(ones_t[:], ones_d[:])
                mask_t = bconst.tile([P, 4, QB], BF16, tag="mask")
                nc.sync.dma_start(mask_t[:], mask_d.rearrange("c p q -> p c q"))
                ident_t = bconst.tile([P, P], BF16, tag="ident")
                nc.sync.dma_start(ident_t[:], ident_d[:])

                att_tiles = []
                for hl in range(HPC):
                    # vk: k-major V tiles via PE transpose, partition=(a'', r')
                    vk_h = hpool.tile([P, NR, P], DT, tag="v")

                    # vk transposes: 4 per q-block, drained by the scalar
                    # engine ahead of its exp work for the block
                    def emit_vk(m, hl=hl, vk_h=vk_h):
                        pst = ps_t.tile([P, P], DT, tag="pt", name="pst")
                        nc.tensor.transpose(
                            pst[:], vt2[hl][:, 8 * m:8 * (m + 1), :], ident_t[:]
                        )
                        nc.scalar.copy(vk_h[:, m, :], pst[:])

                    att_h = attt_pool.tile([P, P, NR], DT, tag="attT")  # [dh, a, r]
                    att_tiles.append(att_h)

                    for qb in range(4):
                        a0 = 32 * qb
                        nk = 4 * (qb + 1)   # k-octets 0..nk-1
                        npair = nk // 2
                        psum_o = ps_o.tile([P, QB], F32, tag="po")
                        psum_l = ps_l.tile([P, QB], F32, tag="pl")
                        ats = [None] * npair

                        def emit_scores(pr, hl=hl, qb=qb, a0=a0, ats=ats):
                            psum_s = ps_s2.tile([P, 2, QB], F32, tag="ps2")
                            for j in (0, 1):
                                m = 2 * pr + j
                                nc.tensor.matmul(
                                    psum_s[:, j, :],
                                    lhsT=kt2[hl][:, 8 * m:8 * (m + 1), :],
                                    rhs=qt2[hl][:, a0:a0 + 32, :],
                                    start=True, stop=True,
                                )
                            atp = atpool.tile([P, 2, QB], DT, tag="at")
                            nc.scalar.activation(atp[:], psum_s[:], AF.Exp, scale=ESC)
                            for j in (0, 1):
                                m = 2 * pr + j
                                if m >= 4 * qb:
                                    nc.vector.tensor_mul(
                                        atp[:, j, :], atp[:, j, :],
                                        mask_t[:, m - 4 * qb, :],
                                    )
                            ats[pr] = atp

                        def emit_ov(pr, nk=nk, ats=ats,
                                    psum_o=psum_o, psum_l=psum_l, vk_h=vk_h):
                            for j in (0, 1):
                                m = 2 * pr + j
                                nc.tensor.matmul(
                                    psum_o[:],
                                    lhsT=vk_h[:, m, :], rhs=ats[pr][:, j, :],
                                    start=(m == 0), stop=(m == nk - 1),
                                )
                                nc.tensor.matmul(
                                    psum_l[:],
                                    lhsT=ones_t[:], rhs=ats[pr][:, j, :],
                                    start=(m == 0), stop=(m == nk - 1),
                                )

                        for m in range(4 * qb, nk):
                            emit_vk(m)
                        emit_scores(0)
                        for pr in range(1, npair):
                            emit_scores(pr)
                            emit_ov(pr - 1)
                        emit_ov(npair - 1)

                        # normalize: att = psum_o * (1/l).  Both PSUM tiles are
                        # copied out to SBUF immediately so their single banks
                        # free before the ~3.3us reciprocal runs (else the next
                        # q-block's first ov/ones matmuls stall the PE).
                        obuf = small.tile([P, QB], F32, tag="obuf")
                        nc.vector.tensor_copy(obuf[:], psum_o[:])
                        lbuf = small.tile([P, QB], F32, tag="lbuf")
                        nc.vector.tensor_copy(lbuf[:], psum_l[:])
                        rcb = small.tile([P, QB], F32, tag="rcb")
                        nc.vector.reciprocal(rcb[:], lbuf[:])
                        nc.vector.tensor_mul(
                            att_h[:, a0:a0 + 32, :],
                            obuf[:].rearrange("p (a r) -> p a r", a=32),
                            rcb[:].rearrange("p (a r) -> p a r", a=32),
                        )

                # ---- phase C: partial output projection yT = WoT.T @ attT ----
                with tc.tile_pool(name="wop", bufs=1) as wop:
                    wot_t = wop.tile([P, HPC, DM], BF16, tag="wo")
                    nc.sync.dma_start(
                        wot_t[:],
                        wot_d.rearrange("(hl p) d -> p hl d", p=P),
                    )
                    att_flat = [
                        t[:].rearrange("p a r -> p (a r)") for t in att_tiles
                    ]
                    for ot in range(N_DM):
                        st = stage.tile([P, S], DT, tag="ystage")
                        for sb in range(4):
                            psum = ps_s2.tile([P, 2, QB], F32, tag="ps2",
                                              name="cpsum")[:, 0, :]
                            for hl in range(HPC):
                                nc.tensor.matmul(
                                    psum,
                                    lhsT=wot_t[:, hl, ot * P:(ot + 1) * P],
                                    rhs=att_flat[hl][:, sb * QB:(sb + 1) * QB],
                                    start=(hl == 0), stop=(hl == HPC - 1),
                                )
                            nc.any.tensor_copy(st[:, sb * QB:(sb + 1) * QB], psum)
                            if ot == N_DM - 1:
                                nc.sync.dma_start(
                                    yt_t3[:, ot, sb * QB:(sb + 1) * QB],
                                    st[:, sb * QB:(sb + 1) * QB],
                                )
                        if ot < N_DM - 1:
                            nc.sync.dma_start(yt_t3[:, ot, :], st[:])

    _split_multi_waits(nc)
    return nc


_NC = None


def _make_masks():
    # a-blocked causal masks for diagonal tiles, (a-outer, r-inner) order:
    # k partition index p = a''*16 + r';  q column index j = a_rel*16 + r
    # allow k <= q:  16*(8*mi + a'') + r'  <=  16*a_rel + r
    k_lin = (16 * np.arange(8)[:, None] + np.arange(NR)[None, :]).reshape(-1)   # 128
    q_lin = (16 * np.arange(32)[:, None] + np.arange(NR)[None, :]).reshape(-1)  # 512
    out = np.empty((4, P, QB), dtype=np.float32)
    for mi in range(4):
        out[mi] = ((k_lin[:, None] + 128 * mi) <= q_lin[None, :]).astype(np.float32)
    return out


def kernel(x, Wq, Wk, Wv, Wo, _want_trace=False, **_trace_kw):
    global _NC
    if _NC is None:
        _NC = _build_nc()
    nc = _NC

    import ml_dtypes
    bf16 = ml_dtypes.bfloat16
    fp8 = ml_dtypes.float8_e4m3fn

    def to8(arr, scale):
        return np.clip(np.asarray(arr, np.float32) * scale, -240.0, 240.0).astype(fp8)

    x = np.asarray(x, dtype=np.float32)
    Wq = np.asarray(Wq, dtype=np.float32)
    Wk = np.asarray(Wk, dtype=np.float32)
    Wv = np.asarray(Wv, dtype=np.float32)
    Wo = np.asarray(Wo, dtype=np.float32)
    wqt8 = to8(np.ascontiguousarray(Wq.T), WSC)
    wkt8 = to8(np.ascontiguousarray(Wk.T), WSC)
    wvt = np.ascontiguousarray(Wv.T).astype(bf16)
    masks = _make_masks().astype(bf16)
    ones = np.ones((P, P), dtype=bf16)
    ident = np.eye(P, dtype=np.float32).astype(bf16)

    in_maps = []
    for c in range(N_CORES):
        b, g = divmod(c, HPC)
        sl = slice(g * DL, (g + 1) * DL)
        xsT = np.ascontiguousarray(x[b, sl, :].T)
        in_maps.append({
            "xsb": xsT.astype(bf16),
            "xs8": to8(xsT, 1.0),
            "wvt": wvt,
            "wkt8": wkt8,
            "wqt8": wqt8,
            "wot": np.ascontiguousarray(Wo[:, sl].T).astype(bf16),
            "maskc": masks,
            "ones": ones,
            "ident": ident,
        })

    res = run_bass_kernel_spmd(
        nc, in_maps, list(range(N_CORES)),
        trace=_want_trace, **_trace_kw,
    )

    y = np.empty((B, S, DM), dtype=np.float32)
    for b in range(B):
        acc = res.results[HPC * b]["yt"].astype(np.float32)
        for g in range(1, HPC):
            acc += res.results[HPC * b + g]["yt"].astype(np.float32)
        y[b] = acc.T
    if _want_trace:
        return y, res
    return y


# revision 32
# speedup vs baseline: 1.0650x; 1.0177x over previous
"""nn_MultiHeadAttention (B=2, S=2048, D=2048, H=16) on 8 NeuronCores.

The reference module splits heads with a plain reshape (no transpose):
    Q = (x @ Wq.T).reshape(B, H, S, Dh)
so head h attends over ROWS [128h, 128h+128) of Qmat = x @ Wq.T, with
attention position s = 16a + r mapping to (row 128h + a, feature slice
[128r, 128r+128)).  The merge DOES transpose (standard), so
    y = sum_h outh @ Wo[:, 128h:128h+128].T.

Sharding: core c handles batch b=c//4 and head-group g=c%4 (heads
4g..4g+3, i.e. Qmat/Kmat/Vmat rows [512g, 512g+512) of its batch).  Each
core computes those projection row-slices (against the FULL Wq/Wk/Wv),
causal attention in the scrambled index space, and a partial output
projection against its column slice of Wo.  The host sums the 4 bf16
partials per batch in fp32.

v2 changes vs the bf16 baseline (385-460us):
  - Q/K projections run in fp8(e4m3) with DoubleRow perf mode: weights
    pre-scaled x32 into fp8 range on the host, the x1024 score scale is
    folded into the softmax exp.  Contraction pairs of 128-deep k-tiles
    go through the PE at 2 fp8 MACs/cell/cycle.
  - The PSUM->SBUF scatter copies of phase A (2B-strided, ~960ns each on
    the scalar engine) are round-robined across scalar and vector so
    neither engine paces the PE.
  - exp runs once per k-octet PAIR over a 2-bank PSUM tile [128, 1024],
    halving activation instruction overhead.
  - startup DMA is chunked so the first matmul waits on ~256KB, not 3MB.
"""

import sys

try:
    import concourse.bass as bass
except ImportError:  # harness may not have the repo on PYTHONPATH
    for p in ("/root/.axon_site", "/root/.axon_site/_ro/trn_rl_repo",
              "/root/.axon_site/_ro/pypackages", "/opt/trn_rl_repo"):
        if p not in sys.path:
            sys.path.append(p)
    import concourse.bass as bass

import numpy as np

import concourse.mybir as mybir
import concourse.tile as tile
from concourse.bass_utils import run_bass_kernel_spmd

F32 = mybir.dt.float32
BF16 = mybir.dt.bfloat16
FP8 = mybir.dt.float8e4
DT = BF16
AF = mybir.ActivationFunctionType
DR = mybir.MatmulPerfMode.DoubleRow

B = 2
S = 2048
DM = 2048
H = 16
DH = 128
N_CORES = 8
HPC = 4                 # heads per core
DL = HPC * DH           # 512: per-core row/col slice width
P = 128
QB = 512                # q-block width = 32 a x 16 r
N_DM = DM // P          # 16 contraction tiles
NR = 16                 # r-stripes per head
WSC = 32.0              # fp8 weight pre-scale for Wq/Wk
ESC = 1.0 / (DH * WSC * WSC)   # exp scale: undo x32 x32 and /128


def _split_multi_waits(nc):
    """This container's walrus rejects >1 sync-wait per instruction.
    Hoist extra waits onto same-engine NoOps inserted just before."""
    ctr = 0
    for f in nc.m.functions:
        for bb in f.blocks:
            insts = bb.instructions
            fixes = []
            for idx, inst in enumerate(insts):
                si = inst.sync_info
                ow = list(si.on_wait) if si and si.on_wait else []
                if len(ow) > 1:
                    fixes.append((idx, inst, ow, si))
            for idx, inst, ow, si in reversed(fixes):
                inst.sync_info = mybir.SyncInfo(on_wait=ow[-1:], on_update=si.on_update)
                for w in reversed(ow[:-1]):
                    ctr += 1
                    nop = mybir.InstNoOp(
                        name=f"I-waitsplit-{ctr}", engine=inst.engine, ins=[], outs=[]
                    )
                    nop.sync_info = mybir.SyncInfo(on_wait=[w], on_update=[])
                    nc.register_instruction(nop, overwrite=True)
                    insts.insert(idx, nop)
    return ctr


def _build_nc():
    nc = bass.Bass(target_bir_lowering=False)

    xsb_d = nc.dram_tensor("xsb", [DM, DL], BF16, kind="ExternalInput")   # x[b,rows].T
    xs8_d = nc.dram_tensor("xs8", [DM, DL], FP8, kind="ExternalInput")
    wvt_d = nc.dram_tensor("wvt", [DM, DM], BF16, kind="ExternalInput")   # Wv.T
    wkt_d = nc.dram_tensor("wkt8", [DM, DM], FP8, kind="ExternalInput")   # Wk.T * 32
    wqt_d = nc.dram_tensor("wqt8", [DM, DM], FP8, kind="ExternalInput")   # Wq.T * 32
    wot_d = nc.dram_tensor("wot", [DL, DM], BF16, kind="ExternalInput")   # Wo[:,sl].T
    mask_d = nc.dram_tensor("maskc", [4, P, QB], BF16, kind="ExternalInput")
    ones_d = nc.dram_tensor("ones", [P, P], BF16, kind="ExternalInput")
    ident_d = nc.dram_tensor("ident", [P, P], BF16, kind="ExternalInput")
    yt_d = nc.dram_tensor("yt", [DM, S], BF16, kind="ExternalOutput")     # partial y[b].T

    yt_t3 = yt_d.rearrange("(o p) s -> p o s", p=P)

    # alternate the strided scatter copies between scalar and vector
    _cp = [0]

    def scatter_copy(dst, src):
        eng = nc.scalar.copy if _cp[0] % 2 == 0 else nc.vector.tensor_copy
        _cp[0] += 1
        eng(dst, src)

    with tile.TileContext(nc) as tc:
        with (
            tc.tile_pool(name="stage", bufs=4) as stage,
            tc.tile_pool(name="small", bufs=2) as small,
            tc.tile_pool(name="proj", bufs=HPC) as proj,
            tc.tile_pool(name="ps_s2", bufs=2, space="PSUM") as ps_s2,
            tc.tile_pool(name="ps_o", bufs=1, space="PSUM") as ps_o,
            tc.tile_pool(name="ps_l", bufs=1, space="PSUM") as ps_l,
            tc.tile_pool(name="ps_t", bufs=2, space="PSUM") as ps_t,
            nc.allow_low_precision(reason="bf16/fp8 attention kernel"),
        ):
            # per-head projection tiles in [dh, a, r] layout; phase A scatters
            # r-stripes into them (strided copies, split across scalar+vector)
            qt2 = [proj.tile([P, P, NR], DT, tag="qt2", name=f"qt2_{i}") for i in range(HPC)]
            kt2 = [proj.tile([P, P, NR], DT, tag="kt2", name=f"kt2_{i}") for i in range(HPC)]
            vt2 = [proj.tile([P, P, NR], DT, tag="vt2", name=f"vt2_{i}") for i in range(HPC)]

            # ---- phase A: projection row-slices straight into SBUF ----
            with (
                tc.tile_pool(name="xpool", bufs=1) as xpool,
                tc.tile_pool(name="wvp", bufs=3) as wvp,
                tc.tile_pool(name="w8p", bufs=4) as w8p,
            ):
                xb_t = xpool.tile([P, N_DM, DL], BF16, tag="xb")
                x8_t = xpool.tile([P, N_DM, DL], FP8, tag="x8")
                xb_t3 = xsb_d.rearrange("(o p) s -> p o s", p=P)
                x8_t3 = xs8_d.rearrange("(o p) s -> p o s", p=P)
                wv_t3 = wvt_d.rearrange("(o p) d -> p o d", p=P)

                # first V weight tile + first half of x ship before anything
                # else so the first matmul can start ~6us in
                wv0 = wvp.tile([P, N_DM, P], BF16, tag="wv")
                nc.sync.dma_start(wv0[:, :4, :], wv_t3[:, :4, 0:P])
                nc.sync.dma_start(xb_t[:, :4, :], xb_t3[:, :4, :])
                nc.sync.dma_start(wv0[:, 4:, :], wv_t3[:, 4:, 0:P])
                nc.sync.dma_start(xb_t[:, 4:8, :], xb_t3[:, 4:8, :])
                nc.sync.dma_start(xb_t[:, 8:12, :], xb_t3[:, 8:12, :])
                nc.sync.dma_start(xb_t[:, 12:, :], xb_t3[:, 12:, :])

                # The fp8-DR passes finish each PSUM group in ~2.3us but the
                # 4 strided scatter copies take ~2us on the drain engines, so
                # a 2-deep PSUM ring has no slack.  Round-robin over 4 DISTINCT
                # tiles drawn from the shared pools (PSUM matmul groups
                # serialize at tile granularity, so two iterations must not
                # share a tile; and reusing phase B's pools here avoids a
                # pool-scope drain barrier at the A->B boundary).

                def a_psum(rt):
                    r = rt % 4
                    if r < 2:
                        return ps_s2.tile([P, 2, QB], F32, tag="ps2",
                                          name="apsum2")[:, 0, :]
                    if r == 2:
                        return ps_o.tile([P, QB], F32, tag="po", name="apsumo")[:]
                    return ps_l.tile([P, QB], F32, tag="pl", name="apsuml")[:]

                # V/K/Q passes interleaved per rt: the fp8-DR K/Q groups
                # finish in ~2.0us but their 4 strided scatter copies cost
                # ~1.9us on the two drain engines -- back-to-back DR groups
                # saturate the drains and any jitter stalls the PE.  Mixing in
                # the 4.1us bf16 V groups keeps drain load at ~70%.
                nc.sync.dma_start(x8_t[:, :8, :], x8_t3[:, :8, :])
                nc.sync.dma_start(x8_t[:, 8:, :], x8_t3[:, 8:, :])
                wk_t3 = wkt_d.rearrange("(o p) d -> p o d", p=P)
                wq_t3 = wqt_d.rearrange("(o p) d -> p o d", p=P)
                apsum_ctr = [0]

                def a_psum():
                    r = apsum_ctr[0] % 4
                    apsum_ctr[0] += 1
                    if r < 2:
                        return ps_s2.tile([P, 2, QB], F32, tag="ps2",
                                          name="apsum2")[:, 0, :]
                    if r == 2:
                        return ps_o.tile([P, QB], F32, tag="po", name="apsumo")[:]
                    return ps_l.tile([P, QB], F32, tag="pl", name="apsuml")[:]

                def emit_v(rt):
                    if rt == 0:
                        w_t = wv0
                    else:
                        w_t = wvp.tile([P, N_DM, P], BF16, tag="wv")
                        nc.sync.dma_start(w_t[:], wv_t3[:, :, rt * P:(rt + 1) * P])
                    psum = a_psum()
                    for dm in range(N_DM):
                        nc.tensor.matmul(
                            psum, lhsT=w_t[:, dm, :], rhs=xb_t[:, dm, :],
                            start=(dm == 0), stop=(dm == N_DM - 1),
                        )
                    for hl in range(HPC):
                        scatter_copy(
                            vt2[hl][:, :, rt], psum[:, hl * P:(hl + 1) * P]
                        )

                def emit_kq(w_t3, dst, rt):
                    w_t = w8p.tile([P, N_DM, P], FP8, tag="w8")
                    nc.sync.dma_start(w_t[:], w_t3[:, :, rt * P:(rt + 1) * P])
                    psum = a_psum()
                    for dp in range(8):
                        nc.tensor.matmul(
                            psum,
                            lhsT=w_t[:, 2 * dp:2 * dp + 2, :],
                            rhs=x8_t[:, 2 * dp:2 * dp + 2, :],
                            start=(dp == 0), stop=(dp == 7),
                            perf_mode=DR,
                        )
                    for hl in range(HPC):
                        scatter_copy(
                            dst[hl][:, :, rt], psum[:, hl * P:(hl + 1) * P]
                        )

                emit_v(0)
                emit_v(1)
                for i in range(14):
                    emit_kq(wk_t3, kt2, i)
                    emit_kq(wq_t3, qt2, i)
                    emit_v(i + 2)
                for rt in (14, 15):
                    emit_kq(wk_t3, kt2, rt)
                    emit_kq(wq_t3, qt2, rt)

            # ---- phase B: attention per head (scrambled index space) ----
            # k-octet m covers kidx = a''*16 + r' (a'' in [8m,8m+8));
            # q-block qb covers qidx = a*16 + r (a in [32qb, 32qb+32)).
            with (
                tc.tile_pool(name="bconst", bufs=1) as bconst,
                tc.tile_pool(name="hpool", bufs=3) as hpool,
                tc.tile_pool(name="atpool", bufs=4) as atpool,
                tc.tile_pool(name="attt", bufs=HPC) as attt_pool,
            ):
                ones_t = bconst.tile([P, P], DT, tag="ones")
                nc.sync.dma_start(ones_t[:], ones_d[:])
                mask_t = bconst.tile([P, 4, QB], BF16, tag="mask")
                nc.sync.dma_start(mask_t[:], mask_d.rearrange("c p q -> p c q"))
                ident_t = bconst.tile([P, P], BF16, tag="ident")
                nc.sync.dma_start(ident_t[:], ident_d[:])

                att_tiles = []
                for hl in range(HPC):
                    # vk: k-major V tiles via PE transpose, partition=(a'', r')
                    vk_h = hpool.tile([P, NR, P], DT, tag="v")

                    # vk transposes: 4 per q-block, drained by the scalar
                    # engine ahead of its exp work for the block
                    def emit_vk(m, hl=hl, vk_h=vk_h):
                        pst = ps_t.tile([P, P], DT, tag="pt", name="pst")
                        nc.tensor.transpose(
                            pst[:], vt2[hl][:, 8 * m:8 * (m + 1), :], ident_t[:]
                        )
                        nc.scalar.copy(vk_h[:, m, :], pst[:])

                    att_h = attt_pool.tile([P, P, NR], DT, tag="attT")  # [dh, a, r]
                    att_tiles.append(att_h)

                    for qb in range(4):
                        a0 = 32 * qb
                        nk = 4 * (qb + 1)   # k-octets 0..nk-1
                        npair = nk // 2
                        psum_o = ps_o.tile([P, QB], F32, tag="po")
                        psum_l = ps_l.tile([P, QB], F32, tag="pl")
                        ats = [None] * npair

                        def emit_scores(pr, hl=hl, qb=qb, a0=a0, ats=ats):
                            psum_s = ps_s2.tile([P, 2, QB], F32, tag="ps2")
                            for j in (0, 1):
                                m = 2 * pr + j
                                nc.tensor.matmul(
                                    psum_s[:, j, :],
                                    lhsT=kt2[hl][:, 8 * m:8 * (m + 1), :],
                                    rhs=qt2[hl][:, a0:a0 + 32, :],
                                    start=True, stop=True,
                                )
                            atp = atpool.tile([P, 2, QB], DT, tag="at")
                            nc.scalar.activation(atp[:], psum_s[:], AF.Exp, scale=ESC)
                            for j in (0, 1):
                                m = 2 * pr + j
                                if m >= 4 * qb:
                                    nc.vector.tensor_mul(
                                        atp[:, j, :], atp[:, j, :],
                                        mask_t[:, m - 4 * qb, :],
                                    )
                            ats[pr] = atp

                        def emit_ov(pr, nk=nk, ats=ats,
                                    psum_o=psum_o, psum_l=psum_l, vk_h=vk_h):
                            for j in (0, 1):
                                m = 2 * pr + j
                                nc.tensor.matmul(
                                    psum_o[:],
                                    lhsT=vk_h[:, m, :], rhs=ats[pr][:, j, :],
                                    start=(m == 0), stop=(m == nk - 1),
                                )
                                nc.tensor.matmul(
                                    psum_l[:],
                                    lhsT=ones_t[:], rhs=ats[pr][:, j, :],
                                    start=(m == 0), stop=(m == nk - 1),
                                )

                        for m in range(4 * qb, nk):
                            emit_vk(m)
                        emit_scores(0)
                        for pr in range(1, npair):
                            emit_scores(pr)
                            emit_ov(pr - 1)
                        emit_ov(npair - 1)

                        # normalize: att = psum_o * (1/l).  Both PSUM tiles are
                        # copied out to SBUF immediately so their single banks
                        # free before the ~3.3us reciprocal runs (else the next
                        # q-block's first ov/ones matmuls stall the PE).
                        obuf = small.tile([P, QB], F32, tag="obuf")
                        nc.vector.tensor_copy(obuf[:], psum_o[:])
                        lbuf = small.tile([P, QB], F32, tag="lbuf")
                        nc.vector.tensor_copy(lbuf[:], psum_l[:])
                        rcb = small.tile([P, QB], F32, tag="rcb")
                        nc.vector.reciprocal(rcb[:], lbuf[:])
                        nc.vector.tensor_mul(
                            att_h[:, a0:a0 + 32, :],
                            obuf[:].rearrange("p (a r) -> p a r", a=32),
                            rcb[:].rearrange("p (a r) -> p a r", a=32),
                        )

                # ---- phase C: partial output projection yT = WoT.T @ attT ----
                with tc.tile_pool(name="wop", bufs=1) as wop:
                    wot_t = wop.tile([P, HPC, DM], BF16, tag="wo")
                    nc.sync.dma_start(
                        wot_t[:],
                        wot_d.rearrange("(hl p) d -> p hl d", p=P),
                    )
                    att_flat = [
                        t[:].rearrange("p a r -> p (a r)") for t in att_tiles
                    ]
                    for ot in range(N_DM):
                        st = stage.tile([P, S], DT, tag="ystage")
                        for sb in range(4):
                            psum = ps_s2.tile([P, 2, QB], F32, tag="ps2",
                                              name="cpsum")[:, 0, :]
                            for hl in range(HPC):
                                nc.tensor.matmul(
                                    psum,
                                    lhsT=wot_t[:, hl, ot * P:(ot + 1) * P],
                                    rhs=att_flat[hl][:, sb * QB:(sb + 1) * QB],
                                    start=(hl == 0), stop=(hl == HPC - 1),
                                )
                            nc.vector.tensor_copy(st[:, sb * QB:(sb + 1) * QB], psum)
                            if ot == N_DM - 1:
                                nc.sync.dma_start(
                                    yt_t3[:, ot, sb * QB:(sb + 1) * QB],
                                    st[:, sb * QB:(sb + 1) * QB],
                                )
                        if ot < N_DM - 1:
                            nc.sync.dma_start(yt_t3[:, ot, :], st[:])

    _split_multi_waits(nc)
    return nc


_NC = None


def _make_masks():
    # a-blocked causal masks for diagonal tiles, (a-outer, r-inner) order:
    # k partition index p = a''*16 + r';  q column index j = a_rel*16 + r
    # allow k <= q:  16*(8*mi + a'') + r'  <=  16*a_rel + r
    k_lin = (16 * np.arange(8)[:, None] + np.arange(NR)[None, :]).reshape(-1)   # 128
    q_lin = (16 * np.arange(32)[:, None] + np.arange(NR)[None, :]).reshape(-1)  # 512
    out = np.empty((4, P, QB), dtype=np.float32)
    for mi in range(4):
        out[mi] = ((k_lin[:, None] + 128 * mi) <= q_lin[None, :]).astype(np.float32)
    return out


def kernel(x, Wq, Wk, Wv, Wo, _want_trace=False, **_trace_kw):
    global _NC
    if _NC is None:
        _NC = _build_nc()
    nc = _NC

    import ml_dtypes
    bf16 = ml_dtypes.bfloat16
    fp8 = ml_dtypes.float8_e4m3fn

    def to8(arr, scale):
        return np.clip(np.asarray(arr, np.float32) * scale, -240.0, 240.0).astype(fp8)

    x = np.asarray(x, dtype=np.float32)
    Wq = np.asarray(Wq, dtype=np.float32)
    Wk = np.asarray(Wk, dtype=np.float32)
    Wv = np.asarray(Wv, dtype=np.float32)
    Wo = np.asarray(Wo, dtype=np.float32)
    wqt8 = to8(np.ascontiguousarray(Wq.T), WSC)
    wkt8 = to8(np.ascontiguousarray(Wk.T), WSC)
    wvt = np.ascontiguousarray(Wv.T).astype(bf16)
    masks = _make_masks().astype(bf16)
    ones = np.ones((P, P), dtype=bf16)
    ident = np.eye(P, dtype=np.float32).astype(bf16)

    in_maps = []
    for c in range(N_CORES):
        b, g = divmod(c, HPC)
        sl = slice(g * DL, (g + 1) * DL)
        xsT = np.ascontiguousarray(x[b, sl, :].T)
        in_maps.append({
            "xsb": xsT.astype(bf16),
            "xs8": to8(xsT, 1.0),
            "wvt": wvt,
            "wkt8": wkt8,
            "wqt8": wqt8,
            "wot": np.ascontiguousarray(Wo[:, sl].T).astype(bf16),
            "maskc": masks,
            "ones": ones,
            "ident": ident,
        })

    res = run_bass_kernel_spmd(
        nc, in_maps, list(range(N_CORES)),
        trace=_want_trace, **_trace_kw,
    )

    y = np.empty((B, S, DM), dtype=np.float32)
    for b in range(B):
        acc = res.results[HPC * b]["yt"].astype(np.float32)
        for g in range(1, HPC):
            acc += res.results[HPC * b + g]["yt"].astype(np.float32)
        y[b] = acc.T
    if _want_trace:
        return y, res
    return y
